# revision 12
# baseline (speedup 1.0000x reference)
"""Trainium2 Bass kernel for nn_DeepUDI (RGAT+GRU message passing), 8-core SPMD.

Sharding: nodes (dim 0) split across 8 cores, 256 nodes = 512 (node,relation)
pairs per core. The full node-state table h=embed[x] is tiny (512KB) so the
neighbor gather is done on host while sharding (graph-parallel, no collectives).

Algebraic restructuring (host-side, exact in fp32):
  attention scores_k = hn_k . (A @ h)  with  A = w @ kw @ qw^T @ w^T  [64,64]
  folds qw,kw (16K elems/pair) into A (4K elems/pair): ~25% less HBM traffic.
  df = (w^T @ (hn^T @ E)) / sum(E);  E = exp(scores)  (|scores|<~30, no max sub)
  GRU gates on DVE with pairs-on-partitions; per-pair matvecs on TensorE with
  per-pair stationary (LDW) + 1-col moving, outputs batched into PSUM columns.
"""

import numpy as np

N, R, K, D, F, D2 = 2048, 2, 32, 64, 64, 128
P_ALL = N * R           # 4096 pairs
NCORES = 8
PPC = P_ALL // NCORES   # 512 pairs/core
NPC = N // NCORES       # 256 nodes/core
TILE = 128              # pairs per DVE tile
GRP = 32                # pairs per TensorE stationary group
NT = PPC // TILE        # 4 tiles/core
NG = PPC // GRP         # 16 groups/core
GPT = TILE // GRP       # 4 groups per tile

_cache = {}


def _build():
    import concourse.mybir as mybir
    import concourse.tile as tile
    from concourse import bacc
    from concourse.masks import make_identity

    fp32 = mybir.dt.float32
    nc = bacc.Bacc(
        "TRN2", target_bir_lowering=False, debug=False, num_devices=NCORES
    )

    # ---- DRAM I/O (per-core shards) ----
    d_wS = nc.dram_tensor("wS", [NG, D, GRP * F], fp32, kind="ExternalInput")
    d_AT = nc.dram_tensor("ATS", [NG, D, GRP * F], fp32, kind="ExternalInput")
    d_hnT = nc.dram_tensor("hnTS", [NG, D, GRP * K], fp32, kind="ExternalInput")
    d_hnS = nc.dram_tensor("hnS", [NG, K, GRP * D], fp32, kind="ExternalInput")
    d_hcT = nc.dram_tensor("hcolT", [NG, D, GRP], fp32, kind="ExternalInput")
    d_Wx = nc.dram_tensor("Wxr", [NT, TILE, 3 * D * F], fp32, kind="ExternalInput")
    d_Wn = nc.dram_tensor("Wnr", [NT, TILE, 3 * F * F], fp32, kind="ExternalInput")
    d_h = nc.dram_tensor("hrow", [NT, TILE, D], fp32, kind="ExternalInput")
    d_b = nc.dram_tensor("brow", [NT, TILE, 3 * F], fp32, kind="ExternalInput")
    d_out = nc.dram_tensor("out", [NT, TILE // 2, F], fp32, kind="ExternalOutput")

    with tile.TileContext(nc) as tc:
        with (
            tc.tile_pool(name="const", bufs=1) as cpool,
            tc.tile_pool(name="stat", bufs=2) as spool,     # TensorE stationaries
            tc.tile_pool(name="big", bufs=2) as bpool,      # Wx/Wn gate tiles
            tc.tile_pool(name="vec", bufs=2) as vpool,      # small vectors
            tc.tile_pool(name="ps", bufs=4, space="PSUM") as pspool,
            tc.tile_pool(name="pst", bufs=2, space="PSUM") as psT,
        ):
            ident = cpool.tile([128, 128], fp32)
            make_identity(nc, ident)
            ones32 = cpool.tile([K, 1], fp32)
            nc.vector.memset(ones32, 1.0)
            one1 = cpool.tile([1, 1], fp32)
            nc.vector.memset(one1, 1.0)

            for t in range(NT):
                # ---- DVE-side tiles ----
                h_row = vpool.tile([TILE, D], fp32, tag="hrow")
                nc.sync.dma_start(out=h_row, in_=d_h[t])
                b_row = vpool.tile([TILE, 3 * F], fp32, tag="brow")
                nc.sync.dma_start(out=b_row, in_=d_b[t])

                # ---- TensorE stages, per group of 32 pairs ----
                s_cat = vpool.tile([1, TILE], fp32, tag="scat")
                df_row_u = vpool.tile([TILE, F], fp32, tag="dfru")
                for gi in range(GPT):
                    g = t * GPT + gi
                    wS = spool.tile([D, GRP * F], fp32, tag="wS")
                    nc.sync.dma_start(out=wS, in_=d_wS[g])
                    ATS = spool.tile([D, GRP * F], fp32, tag="ATS")
                    nc.sync.dma_start(out=ATS, in_=d_AT[g])
                    hnT = spool.tile([D, GRP * K], fp32, tag="hnT")
                    nc.sync.dma_start(out=hnT, in_=d_hnT[g])
                    hnS = spool.tile([K, GRP * D], fp32, tag="hnS")
                    nc.sync.dma_start(out=hnS, in_=d_hnS[g])
                    hcT = spool.tile([D, GRP], fp32, tag="hcT")
                    nc.sync.dma_start(out=hcT, in_=d_hcT[g])

                    # u = A @ h   -> PSUM [D, GRP]
                    u_ps = pspool.tile([D, GRP], fp32, tag="gps")
                    for i in range(GRP):
                        nc.tensor.matmul(
                            u_ps[:, i : i + 1],
                            ATS[:, i * F : (i + 1) * F],
                            hcT[:, i : i + 1],
                            start=True, stop=True,
                        )
                    u_sb = vpool.tile([D, GRP], fp32, tag="usb")
                    nc.vector.tensor_copy(u_sb, u_ps)

                    # scores = hn @ u -> PSUM [K, GRP]
                    sc_ps = pspool.tile([K, GRP], fp32, tag="gps")
                    for i in range(GRP):
                        nc.tensor.matmul(
                            sc_ps[:, i : i + 1],
                            hnT[:, i * K : (i + 1) * K],
                            u_sb[:, i : i + 1],
                            start=True, stop=True,
                        )
                    # E = exp(scores) (unnormalized)
                    E_sb = vpool.tile([K, GRP], fp32, tag="esb")
                    nc.scalar.activation(
                        out=E_sb, in_=sc_ps,
                        func=mybir.ActivationFunctionType.Exp,
                    )
                    # s = sum_k E -> [1, GRP]
                    s_ps = psT.tile([1, GRP], fp32, tag="tps")
                    nc.tensor.matmul(s_ps, ones32, E_sb, start=True, stop=True)
                    nc.vector.tensor_copy(
                        s_cat[:, gi * GRP : (gi + 1) * GRP], s_ps
                    )

                    # g = hn^T @ E -> [D, GRP]
                    g_ps = pspool.tile([D, GRP], fp32, tag="gps")
                    for i in range(GRP):
                        nc.tensor.matmul(
                            g_ps[:, i : i + 1],
                            hnS[:, i * D : (i + 1) * D],
                            E_sb[:, i : i + 1],
                            start=True, stop=True,
                        )
                    g_sb = vpool.tile([D, GRP], fp32, tag="gsb")
                    nc.vector.tensor_copy(g_sb, g_ps)

                    # df_u = w^T @ g -> [F, GRP]
                    df_ps = pspool.tile([F, GRP], fp32, tag="gps")
                    for i in range(GRP):
                        nc.tensor.matmul(
                            df_ps[:, i : i + 1],
                            wS[:, i * F : (i + 1) * F],
                            g_sb[:, i : i + 1],
                            start=True, stop=True,
                        )
                    df_sb = vpool.tile([F, GRP], fp32, tag="dfsb")
                    nc.vector.tensor_copy(df_sb, df_ps)
                    dft_ps = psT.tile([GRP, F], fp32, tag="tps")
                    nc.tensor.transpose(dft_ps, df_sb, ident[:F, :F])
                    nc.vector.tensor_copy(
                        df_row_u[gi * GRP : (gi + 1) * GRP, :], dft_ps
                    )

                # ---- 1/s as a column [TILE, 1]; df to rows, normalized ----
                rs_cat = vpool.tile([1, TILE], fp32, tag="rscat")
                nc.vector.reciprocal(rs_cat, s_cat)
                rs_ps = psT.tile([TILE, 1], fp32, tag="tps")
                nc.tensor.matmul(rs_ps, rs_cat, one1, start=True, stop=True)
                rs_col = vpool.tile([TILE, 1], fp32, tag="rscol")
                nc.vector.tensor_copy(rs_col, rs_ps)
                df_row = vpool.tile([TILE, F], fp32, tag="dfrow")
                nc.vector.tensor_scalar_mul(df_row, df_row_u, rs_col)

                # ---- GRU gates on DVE (pairs on partitions) ----
                def matvec_row(w_dram_col0, nelem_in, vec_row, tag):
                    """out[p,g] = sum_f vec[p,f] * W[p,f,g]  via TT mul + reduce."""
                    Wt = bpool.tile([TILE, nelem_in * F], fp32, tag="gateW")
                    nc.sync.dma_start(out=Wt, in_=w_dram_col0)
                    prod = bpool.tile([TILE, nelem_in * F], fp32, tag="gateP")
                    nc.vector.tensor_tensor(
                        out=prod.rearrange("p (f g) -> p f g", f=nelem_in),
                        in0=Wt.rearrange("p (f g) -> p f g", f=nelem_in),
                        in1=vec_row.to_broadcast([TILE, nelem_in, F]),
                        op=mybir.AluOpType.mult,
                    )
                    red = vpool.tile([TILE, F], fp32, tag=tag)
                    nc.vector.tensor_reduce(
                        out=red,
                        in_=prod.rearrange("p (f g) -> p g f", f=nelem_in),
                        axis=mybir.AxisListType.X,
                        op=mybir.AluOpType.add,
                    )
                    return red

                Wx_ap = d_Wx[t].rearrange("p (j e) -> p j e", j=3)
                Wn_ap = d_Wn[t].rearrange("p (j e) -> p j e", j=3)
                X0 = matvec_row(Wx_ap[:, 0, :], D, h_row, "X0")
                X1 = matvec_row(Wx_ap[:, 1, :], D, h_row, "X1")
                X2 = matvec_row(Wx_ap[:, 2, :], D, h_row, "X2")
                A0 = matvec_row(Wn_ap[:, 0, :], F, df_row, "A0")
                A1 = matvec_row(Wn_ap[:, 1, :], F, df_row, "A1")

                def gate(x, a, j, func):
                    pre = vpool.tile([TILE, F], fp32, tag=f"pre{j}")
                    nc.vector.tensor_add(pre, x, a)
                    nc.vector.tensor_add(pre, pre, b_row[:, j * F : (j + 1) * F])
                    o = vpool.tile([TILE, F], fp32, tag=f"gate{j}")
                    nc.scalar.activation(out=o, in_=pre, func=func)
                    return o

                Sig = mybir.ActivationFunctionType.Sigmoid
                Rg = gate(X0, A0, 0, Sig)
                Z = gate(X1, A1, 1, Sig)
                rdf = vpool.tile([TILE, F], fp32, tag="rdf")
                nc.vector.tensor_mul(rdf, Rg, df_row)
                A2 = matvec_row(Wn_ap[:, 2, :], F, rdf, "A2")
                Hc = gate(X2, A2, 2, mybir.ActivationFunctionType.Tanh)

                # gru = Hc + Z*(df - Hc)
                gru = vpool.tile([TILE, F], fp32, tag="gru")
                nc.vector.tensor_sub(gru, df_row, Hc)
                nc.vector.tensor_mul(gru, gru, Z)
                nc.vector.tensor_add(gru, gru, Hc)

                # ---- mean over r, tanh, transpose to [nodes, F] ----
                gruT_ps = psT.tile([F, TILE], fp32, tag="tps")
                nc.tensor.transpose(gruT_ps, gru, ident[:TILE, :TILE])
                gruT = vpool.tile([F, TILE], fp32, tag="gruT")
                nc.vector.tensor_copy(gruT, gruT_ps)
                tcol = vpool.tile([F, TILE // 2], fp32, tag="tcol")
                nc.vector.tensor_add(
                    tcol,
                    gruT.rearrange("f (n r) -> f r n", r=2)[:, 0, :],
                    gruT.rearrange("f (n r) -> f r n", r=2)[:, 1, :],
                )
                ocolT = vpool.tile([F, TILE // 2], fp32, tag="ocolT")
                nc.scalar.activation(
                    out=ocolT, in_=tcol,
                    func=mybir.ActivationFunctionType.Tanh, scale=0.5,
                )
                out_ps = psT.tile([TILE // 2, F], fp32, tag="tps")
                nc.tensor.transpose(out_ps, ocolT, ident[:F, :F])
                out_sb = vpool.tile([TILE // 2, F], fp32, tag="outsb")
                nc.vector.tensor_copy(out_sb, out_ps)
                nc.sync.dma_start(out=d_out[t], in_=out_sb)

    nc.compile()
    return nc


def _prep(inputs):
    x = np.asarray(inputs["x"]).astype(np.int64)
    nb = np.asarray(inputs["neighbors"]).astype(np.int64)
    embed = np.asarray(inputs["embed"], dtype=np.float32)
    w = np.asarray(inputs["w"], dtype=np.float32).reshape(P_ALL, D, F)
    qw = np.asarray(inputs["qw"], dtype=np.float32).reshape(P_ALL, F, D2)
    kw = np.asarray(inputs["kw"], dtype=np.float32).reshape(P_ALL, F, D2)
    Wx = np.asarray(inputs["Wx"], dtype=np.float32).reshape(P_ALL, 3 * D * F)
    Wn = np.asarray(inputs["Wn"], dtype=np.float32).reshape(P_ALL, 3 * F * F)
    b = (
        np.asarray(inputs["bx"], dtype=np.float32)
        + np.asarray(inputs["bn"], dtype=np.float32)
    ).reshape(P_ALL, 3 * F)

    h = embed[x]                                   # [N, D]
    hv = h[np.repeat(np.arange(N), R)]             # [P, D]
    hn = h[nb.reshape(P_ALL, K)]                   # [P, K, D]
    A = w @ kw @ qw.transpose(0, 2, 1) @ w.transpose(0, 2, 1)  # [P, D, D]

    in_maps = []
    for c in range(NCORES):
        s = slice(c * PPC, (c + 1) * PPC)
        w_c, A_c, hn_c, hv_c = w[s], A[s], hn[s], hv[s]
        m = {
            # [NG, GRP, D, F] -> [NG, D, GRP*F]
            "wS": np.ascontiguousarray(
                w_c.reshape(NG, GRP, D, F).transpose(0, 2, 1, 3).reshape(NG, D, GRP * F)
            ),
            # A^T: lhsT[i, o] = A[o, i]
            "ATS": np.ascontiguousarray(
                A_c.reshape(NG, GRP, D, D).transpose(0, 3, 1, 2).reshape(NG, D, GRP * D)
            ),
            # hn^T: [d, (p k)]
            "hnTS": np.ascontiguousarray(
                hn_c.reshape(NG, GRP, K, D).transpose(0, 3, 1, 2).reshape(NG, D, GRP * K)
            ),
            # hn: [k, (p d)]
            "hnS": np.ascontiguousarray(
                hn_c.reshape(NG, GRP, K, D).transpose(0, 2, 1, 3).reshape(NG, K, GRP * D)
            ),
            "hcolT": np.ascontiguousarray(
                hv_c.reshape(NG, GRP, D).transpose(0, 2, 1)
            ),
            "Wxr": np.ascontiguousarray(Wx[s].reshape(NT, TILE, 3 * D * F)),
            "Wnr": np.ascontiguousarray(Wn[s].reshape(NT, TILE, 3 * F * F)),
            "hrow": np.ascontiguousarray(hv_c.reshape(NT, TILE, D)),
            "brow": np.ascontiguousarray(b[s].reshape(NT, TILE, 3 * F)),
        }
        in_maps.append(m)
    return in_maps


def kernel(**inputs):
    from concourse.bass_utils import run_bass_kernel_spmd

    if "nc" not in _cache:
        _cache["nc"] = _build()
    in_maps = _prep(inputs)
    res = run_bass_kernel_spmd(_cache["nc"], in_maps, list(range(NCORES)))
    outs = [res.results[c]["out"].reshape(NPC, F) for c in range(NCORES)]
    return np.concatenate(outs, axis=0)
